# revision 21
# baseline (speedup 1.0000x reference)
"""Bahdanau-attention kernel for 8 TRN2 NeuronCores.

Reference computation (B=32, S=2048, H=1024):
    eo   = encoder_outputs.transpose(1,0,2)            # [B,S,H]
    z    = hidden @ W[:, :H].T + eo @ W[:, H:].T + b   # [B,S,H]  (split concat)
    s    = tanh(z)
    sc   = einsum('bsh,h->bs', s, v)
    sc   = where(mask, -1e9, sc); softmax over S       # [B,1,S]

Device work is the irreducible nonlinear core: z8 = w8 @ e8 (fp8 e4m3
DoubleRow matmuls, 2 k-tiles per instruction at double rate), tanh with
the hidden-path bias fused (ScalarE), the v-weighted accumulate
(VectorE, bf16 2x) reduced across partitions by a ones-matmul, then a
masked exp + normalize.

Everything linear in the inputs is precomputed exactly on the host and
injected as bias rows:
  * pre[b,h]  = hidden @ Wh^T + b          (tanh per-partition bias)
  * c[b,s]    = u.eo - u8.e8  with u = We^T v, u8 = dequant(w8)^T vb
    -- the exact linear error of the fp8 z-path, added to the score row
    (folded into the same row that kills padding columns with -1e30).
score = v.tanh(z8) + c reproduces the reference to ~1e-2 of max output.

Mask-skip: masked positions softmax to exactly 0 in fp32, so only
unmasked columns are packed (host gather), computed, and scattered back.

Sharding: data-parallel over batch, 4 batches per core, no collectives.
"""

import sys

if "/opt/trn_rl_repo" not in sys.path:
    sys.path.insert(0, "/opt/trn_rl_repo")

import numpy as np

B, S, H = 32, 2048, 1024
NCORES = 8
BL = B // NCORES          # batches per core = 4
P = 128                   # partitions
KT = H // P               # k-tiles over the contraction dim = 8
KP = KT // 2              # DoubleRow k-tile pairs = 4
HT = H // P               # h-tiles over the attn output dim = 8
SE = 16.0                 # eo fp8 scale
SW = 32.0                 # We fp8 scale
ZS = 1.0 / (SE * SW)      # psum -> z units

MAXC = 512                # max chunk width (psum bank, fp32)
NWARM = 13                # PE warmup matmuls (p-state ramp + head DMA)

_compiled = {}


def _chunks(cap, maxc=MAXC):
    nch = -(-cap // maxc)
    base = -(-cap // (nch * 8)) * 8
    widths = [base] * (nch - 1)
    widths.append(cap - base * (nch - 1))
    assert all(0 < w <= maxc for w in widths) and sum(widths) == cap
    return widths


def _build(cap):
    import concourse.mybir as mybir
    from concourse import tile, bacc
    from concourse.tile import add_dep_helper

    f32 = mybir.dt.float32
    bf16 = mybir.dt.bfloat16
    fp8 = mybir.dt.float8e4
    AF = mybir.ActivationFunctionType
    ALU = mybir.AluOpType
    AX = mybir.AxisListType
    DR = mybir.MatmulPerfMode.DoubleRow

    widths0 = _chunks(cap)
    nch = len(widths0)
    # last batch: biggest chunks first, minimal final chunk -> short tail
    wlast = [MAXC] * (nch - 1) + [cap - MAXC * (nch - 1)]
    if min(wlast) <= 0:
        wlast = widths0
    bwidths = [widths0] * (BL - 1) + [wlast]
    boffs = [[sum(w[:i]) for i in range(nch)] for w in bwidths]

    nc = bacc.Bacc("TRN2", target_bir_lowering=False, debug=False,
                   num_devices=NCORES)

    eo8 = nc.dram_tensor("eo8", [BL, P, KP, 2, cap], fp8, kind="ExternalInput")
    w8st = nc.dram_tensor("w8st", [P, HT, KP, 2, P], fp8, kind="ExternalInput")
    vsc = nc.dram_tensor("vsc", [P, HT], f32, kind="ExternalInput")
    prer = nc.dram_tensor("prer", [P, HT * BL], f32, kind="ExternalInput")
    padc = nc.dram_tensor("padc", [1, BL * cap], f32, kind="ExternalInput")
    eout = nc.dram_tensor("eout", [BL, cap], f32, kind="ExternalOutput")
    sout = nc.dram_tensor("sout", [1, BL * nch], f32, kind="ExternalOutput")

    with tile.TileContext(nc) as tc:
        with (
            tc.tile_pool(name="const", bufs=1) as const,
            tc.tile_pool(name="eo", bufs=2) as eo_pool,
            tc.tile_pool(name="tpool", bufs=18) as t_pool,
            tc.tile_pool(name="accpool", bufs=5) as acc_pool,
            tc.tile_pool(name="scpool", bufs=3) as sc_pool,
            tc.tile_pool(name="psz", bufs=5, space="PSUM") as psum_z,
            tc.tile_pool(name="pss", bufs=3, space="PSUM") as psum_s,
        ):
            # --- weights first, split per-hh so z(0) can start after 128KB;
            # tiny consts ride behind the first weight slices. ---
            w8_sb = const.tile([P, HT, KP, 2, P], fp8)
            for hh in range(HT):
                nc.sync.dma_start(w8_sb[:, hh], w8st[:, hh])
            eo_first = eo_pool.tile([P, KP, 2, cap], fp8, tag="eo")
            nc.gpsimd.dma_start(eo_first[:], eo8[0])

            vsc_sb = const.tile([P, HT], f32)
            nc.sync.dma_start(vsc_sb[:], vsc[:, :])
            vsc8_sb = const.tile([P, HT], bf16)
            nc.vector.tensor_copy(vsc8_sb[:], vsc_sb[:])
            pre_sb = const.tile([P, HT * BL], f32)
            nc.sync.dma_start(pre_sb[:], prer[:, :])
            padc_sb = const.tile([1, BL * cap], f32)
            nc.sync.dma_start(padc_sb[:], padc[:, :])

            ones_sb = const.tile([P, 1], bf16)
            nc.any.memset(ones_sb[:], 1.0)
            junk = const.tile([P, MAXC], bf16)
            nc.vector.tensor_copy(junk[:, 0:1], ones_sb[:])

            # PE warmup: ride out the p-state ramp while the head DMAs land
            wps = psum_z.tile([P, MAXC], f32, tag="psz")
            for w in range(NWARM):
                nc.tensor.matmul(wps[:], junk[:, 0:P], junk[:],
                                 start=(w == 0), stop=(w == NWARM - 1),
                                 skip_group_check=True)

            sums = const.tile([1, BL * nch], f32)
            e_rows = [const.tile([1, cap], f32, name=f"e_row{i}")
                      for i in range(BL)]

            # (acc tile, batch, chunk) awaiting their ones-matmul reduce;
            # flushed inside the NEXT chunk's z-groups so the PE never
            # stalls on the tanh/vector chain.
            pending = []

            def flush_pending():
                for acc, pb, pci in pending:
                    pwc = bwidths[pb][pci]
                    pc0 = boffs[pb][pci]
                    pss = psum_s.tile([1, MAXC], f32, tag="pss")
                    nc.tensor.matmul(pss[:1, :pwc], ones_sb[:], acc[:],
                                     start=True, stop=True,
                                     skip_group_check=True)
                    sc_m = sc_pool.tile([1, pwc], f32, tag="sc")
                    off = pb * cap + pc0
                    nc.vector.tensor_tensor(sc_m[:], pss[:1, :pwc],
                                            padc_sb[:, off:off + pwc],
                                            ALU.add)
                    idx = pb * nch + pci
                    nc.scalar.activation(
                        e_rows[pb][:, pc0:pc0 + pwc], sc_m[:], AF.Exp,
                        accum_out=sums[:, idx:idx + 1])
                    if pci == nch - 1:
                        nc.sync.dma_start(eout[pb:pb + 1, :], e_rows[pb][:])
                pending.clear()

            for b in range(BL):
                if b == 0:
                    eo_sb = eo_first
                else:
                    eo_sb = eo_pool.tile([P, KP, 2, cap], fp8, tag="eo")
                    nc.gpsimd.dma_start(eo_sb[:], eo8[b])
                order = ([(hh, ci) for hh in range(HT) for ci in range(nch)]
                         if b == 0 else
                         [(hh, ci) for ci in range(nch) for hh in range(HT)])
                accs = {}
                t8s_m = {}
                pss_m = {}
                for hh, ci in order:
                    c0, wc = boffs[b][ci], bwidths[b][ci]
                    cs = slice(c0, c0 + wc)
                    tail = (b == BL - 1 and ci >= nch - 2)
                    if hh == 0:
                        if tail:
                            pss_m[ci] = psum_s.tile([1, MAXC], f32,
                                                    tag="pss", name="pss_t")
                            t8s_m[ci] = []
                        else:
                            accs[ci] = acc_pool.tile([P, wc], bf16,
                                                     tag="acc", name="acc")
                    zp = psum_z.tile([P, wc], f32, tag="psz")
                    for j in range(KP):
                        nc.tensor.matmul(
                            zp[:], w8_sb[:, hh, j, :, :],
                            eo_sb[:, j, :, cs], start=(j == 0),
                            stop=(j == KP - 1), perf_mode=DR)
                    if hh == 2 and pending:
                        flush_pending()
                    t8 = t_pool.tile([P, wc], bf16, tag="t")
                    nc.scalar.activation(
                        t8[:], zp[:], AF.Tanh, scale=ZS,
                        bias=pre_sb[:, hh * BL + b:hh * BL + b + 1])
                    if tail:
                        pss_t = pss_m[ci]
                        t8s = t8s_m[ci]
                        t8s.append(t8)
                        if hh >= 2:
                            nc.tensor.matmul(
                                pss_t[:1, :wc],
                                vsc8_sb[:, hh - 2:hh - 2 + 1],
                                t8s[hh - 2][:], start=(hh == 2),
                                stop=False, skip_group_check=True)
                    elif hh == 0:
                        nc.vector.tensor_scalar(accs[ci][:], t8[:],
                                                vsc_sb[:, 0:1], None,
                                                ALU.mult)
                    else:
                        tv = t_pool.tile([P, wc], bf16, tag="tv")
                        nc.vector.tensor_scalar(tv[:], t8[:],
                                                vsc_sb[:, hh:hh + 1],
                                                None, ALU.mult)
                        nc.vector.tensor_tensor(accs[ci][:], accs[ci][:],
                                                tv[:], ALU.add)
                    if hh == HT - 1:
                        if tail:
                            for h2 in range(HT - 2, HT):
                                nc.tensor.matmul(
                                    pss_t[:1, :wc], vsc8_sb[:, h2:h2 + 1],
                                    t8s[h2][:], start=False,
                                    stop=(h2 == HT - 1),
                                    skip_group_check=True)
                            sc_m = sc_pool.tile([1, wc], f32, tag="sc")
                            off = b * cap + c0
                            nc.vector.tensor_tensor(sc_m[:], pss_t[:1, :wc],
                                                    padc_sb[:, off:off + wc],
                                                    ALU.add)
                            idx = b * nch + ci
                            nc.scalar.activation(
                                e_rows[b][:, c0:c0 + wc], sc_m[:], AF.Exp,
                                accum_out=sums[:, idx:idx + 1])
                            if ci == nch - 1:
                                nc.sync.dma_start(eout[b:b + 1, :],
                                                  e_rows[b][:])
                        else:
                            pending.append((accs[ci], b, ci))
            flush_pending()
            nc.sync.dma_start(sout[:, :], sums[:])

    nc.compile()
    return nc


def _get_nc(cap=1072):
    if cap not in _compiled:
        _compiled[cap] = _build(cap)
    return _compiled[cap]


def _prep(hidden, encoder_outputs, encoder_mask, W, b, v):
    """Host-side packing/quantization. Returns (in_maps, scatter_info)."""
    import ml_dtypes

    bf16 = ml_dtypes.bfloat16
    f8 = ml_dtypes.float8_e4m3

    hidden = np.asarray(hidden, dtype=np.float32)
    eo = np.asarray(encoder_outputs, dtype=np.float32)      # [S, B, H]
    W = np.asarray(W, dtype=np.float32)
    bias = np.asarray(b, dtype=np.float32)
    v = np.asarray(v, dtype=np.float32)
    mask = np.asarray(encoder_mask).reshape(B, S)

    Wh, We = W[:, :H], W[:, H:]

    w8 = (We * SW).astype(f8)
    w8f = w8.astype(np.float32)
    vb = v.astype(bf16).astype(np.float32)
    u = (We.T @ v).astype(np.float32)            # exact linear weights
    u8 = (w8f / SW).T @ vb                       # device linear weights

    pre = hidden @ Wh.T + bias                   # [B, H] exact hidden path

    # per-batch unmasked indices, uniform padded capacity
    idxs = [np.nonzero(mask[gb] == 0)[0] for gb in range(B)]
    ns = [len(ix) for ix in idxs]
    cap = max(8, -(-max(max(ns), 1) // 8) * 8)

    # stationary layout: k = j*256 + i*128 + p
    w8st = np.ascontiguousarray(
        w8.T.reshape(KP, 2, P, HT, P).transpose(2, 3, 0, 1, 4))
    vsc = np.ascontiguousarray(
        v.astype(bf16).astype(np.float32).reshape(HT, P).T)

    in_maps = []
    for c in range(NCORES):
        eo8c = np.zeros((BL, P, KP, 2, cap), dtype=f8)
        padc = np.zeros((BL, cap), dtype=np.float32)
        for bl in range(BL):
            gb = c * BL + bl
            ix = idxs[gb]
            n = len(ix)
            ecols = np.ascontiguousarray(eo[ix, gb, :].T)   # [H, n]
            e8 = (ecols * SE).astype(f8)
            eo8c[bl, :, :, :, :n] = e8.reshape(
                KP, 2, P, n).transpose(2, 0, 1, 3)
            # exact linear correction of the fp8 z-path
            padc[bl, :n] = u @ ecols - (u8 @ e8.astype(np.float32)) / SE
            padc[bl, n:] = -1e30

        pre_c = pre[c * BL:(c + 1) * BL]                    # [BL, H]
        pre_r = np.ascontiguousarray(
            pre_c.reshape(BL, HT, P).transpose(2, 1, 0).reshape(P, HT * BL))
        in_maps.append({
            "eo8": eo8c,
            "w8st": w8st,
            "vsc": vsc,
            "prer": pre_r,
            "padc": padc.reshape(1, BL * cap),
        })
    return in_maps, (idxs, ns, cap)


def run(hidden, encoder_outputs, encoder_mask, W, b, v, trace=False):
    from concourse.bass_utils import run_bass_kernel_spmd

    in_maps, (idxs, ns, cap) = _prep(
        hidden, encoder_outputs, encoder_mask, W, b, v)
    nc = _get_nc(cap)
    res = run_bass_kernel_spmd(nc, in_maps, core_ids=list(range(NCORES)),
                               trace=trace)
    nch = len(_chunks(cap))
    full = np.zeros((B, S), dtype=np.float32)
    for c in range(NCORES):
        e = res.results[c]["eout"]
        sm = res.results[c]["sout"].reshape(BL, nch).sum(axis=1)
        for bl in range(BL):
            gb = c * BL + bl
            if ns[gb] == 0:
                full[gb, :] = 1.0 / S     # all masked: softmax is uniform
            else:
                full[gb, idxs[gb]] = e[bl, :ns[gb]] / sm[bl]
    return full.reshape(B, 1, S), res


def kernel(hidden, encoder_outputs, encoder_mask, W, b, v):
    out, _ = run(hidden, encoder_outputs, encoder_mask, W, b, v, trace=False)
    return out


# revision 24
# speedup vs baseline: 1.0028x; 1.0028x over previous
"""Bahdanau-attention kernel for 8 TRN2 NeuronCores.

Reference computation (B=32, S=2048, H=1024):
    eo   = encoder_outputs.transpose(1,0,2)            # [B,S,H]
    z    = hidden @ W[:, :H].T + eo @ W[:, H:].T + b   # [B,S,H]  (split concat)
    s    = tanh(z)
    sc   = einsum('bsh,h->bs', s, v)
    sc   = where(mask, -1e9, sc); softmax over S       # [B,1,S]

Device work is the irreducible nonlinear core: z8 = w8 @ e8 (fp8 e4m3
DoubleRow matmuls, 2 k-tiles per instruction at double rate), tanh with
the hidden-path bias fused (ScalarE), the v-weighted accumulate
(VectorE, bf16 2x) reduced across partitions by a ones-matmul, then a
masked exp + normalize.

Everything linear in the inputs is precomputed exactly on the host and
injected as bias rows:
  * pre[b,h]  = hidden @ Wh^T + b          (tanh per-partition bias)
  * c[b,s]    = u.eo - u8.e8  with u = We^T v, u8 = dequant(w8)^T vb
    -- the exact linear error of the fp8 z-path, added to the score row
    (folded into the same row that kills padding columns with -1e30).
score = v.tanh(z8) + c reproduces the reference to ~1e-2 of max output.

Mask-skip: masked positions softmax to exactly 0 in fp32, so only
unmasked columns are packed (host gather), computed, and scattered back.

Sharding: data-parallel over batch, 4 batches per core, no collectives.
"""

import sys

if "/opt/trn_rl_repo" not in sys.path:
    sys.path.insert(0, "/opt/trn_rl_repo")

import numpy as np

B, S, H = 32, 2048, 1024
NCORES = 8
BL = B // NCORES          # batches per core = 4
P = 128                   # partitions
KT = H // P               # k-tiles over the contraction dim = 8
KP = KT // 2              # DoubleRow k-tile pairs = 4
HT = H // P               # h-tiles over the attn output dim = 8
SE = 16.0                 # eo fp8 scale
SW = 32.0                 # We fp8 scale
ZS = 1.0 / (SE * SW)      # psum -> z units

MAXC = 512                # max chunk width (psum bank, fp32)
NWARM = 13                # PE warmup matmuls (p-state ramp + head DMA)

_compiled = {}


def _chunks(cap, maxc=MAXC):
    nch = -(-cap // maxc)
    base = -(-cap // (nch * 8)) * 8
    widths = [base] * (nch - 1)
    widths.append(cap - base * (nch - 1))
    assert all(0 < w <= maxc for w in widths) and sum(widths) == cap
    return widths


def _build(cap):
    import concourse.mybir as mybir
    from concourse import tile, bacc
    from concourse.tile import add_dep_helper

    f32 = mybir.dt.float32
    bf16 = mybir.dt.bfloat16
    fp8 = mybir.dt.float8e4
    AF = mybir.ActivationFunctionType
    ALU = mybir.AluOpType
    AX = mybir.AxisListType
    DR = mybir.MatmulPerfMode.DoubleRow

    widths0 = _chunks(cap)
    nch = len(widths0)
    # last batch: biggest chunks first, minimal final chunk -> short tail
    wlast = [MAXC] * (nch - 1) + [cap - MAXC * (nch - 1)]
    if min(wlast) <= 0:
        wlast = widths0
    bwidths = [widths0] * (BL - 1) + [wlast]
    boffs = [[sum(w[:i]) for i in range(nch)] for w in bwidths]

    nc = bacc.Bacc("TRN2", target_bir_lowering=False, debug=False,
                   num_devices=NCORES)

    eo8 = nc.dram_tensor("eo8", [BL, P, KP, 2, cap], fp8, kind="ExternalInput")
    w8st = nc.dram_tensor("w8st", [P, HT, KP, 2, P], fp8, kind="ExternalInput")
    vsc = nc.dram_tensor("vsc", [P, HT], f32, kind="ExternalInput")
    prer = nc.dram_tensor("prer", [P, HT * BL], f32, kind="ExternalInput")
    padc = nc.dram_tensor("padc", [1, BL * cap], f32, kind="ExternalInput")
    eout = nc.dram_tensor("eout", [BL, cap], f32, kind="ExternalOutput")
    sout = nc.dram_tensor("sout", [1, BL * nch], f32, kind="ExternalOutput")

    with tile.TileContext(nc) as tc:
        with (
            tc.tile_pool(name="const", bufs=1) as const,
            tc.tile_pool(name="eo", bufs=2) as eo_pool,
            tc.tile_pool(name="tpool", bufs=18) as t_pool,
            tc.tile_pool(name="accpool", bufs=5) as acc_pool,
            tc.tile_pool(name="scpool", bufs=3) as sc_pool,
            tc.tile_pool(name="psz", bufs=5, space="PSUM") as psum_z,
            tc.tile_pool(name="pss", bufs=3, space="PSUM") as psum_s,
        ):
            # --- batch-0 eo split across both rings ahead of everything,
            # then weights per-hh on sync so z(0) can start ~10us; tiny
            # consts ride behind. ---
            w8_sb = const.tile([P, HT, KP, 2, P], fp8)
            eo_first = eo_pool.tile([P, KP, 2, cap], fp8, tag="eo")
            for hh in range(HT):
                nc.sync.dma_start(w8_sb[:, hh], w8st[:, hh])
            nc.gpsimd.dma_start(eo_first[:], eo8[0])

            vsc_sb = const.tile([P, HT], f32)
            nc.sync.dma_start(vsc_sb[:], vsc[:, :])
            vsc8_sb = const.tile([P, HT], bf16)
            nc.vector.tensor_copy(vsc8_sb[:], vsc_sb[:])
            pre_sb = const.tile([P, HT * BL], f32)
            nc.sync.dma_start(pre_sb[:], prer[:, :])
            padc_sb = const.tile([1, BL * cap], f32)
            nc.sync.dma_start(padc_sb[:], padc[:, :])

            ones_sb = const.tile([P, 1], bf16)
            nc.any.memset(ones_sb[:], 1.0)
            junk = const.tile([P, MAXC], bf16)
            nc.vector.tensor_copy(junk[:, 0:1], ones_sb[:])

            # PE warmup: ride out the p-state ramp while the head DMAs land
            wps = psum_z.tile([P, MAXC], f32, tag="psz")
            for w in range(NWARM):
                nc.tensor.matmul(wps[:], junk[:, 0:P], junk[:],
                                 start=(w == 0), stop=(w == NWARM - 1),
                                 skip_group_check=True)

            sums = const.tile([1, BL * nch], f32)
            e_rows = [const.tile([1, cap], f32, name=f"e_row{i}")
                      for i in range(BL)]

            # (acc tile, batch, chunk) awaiting their ones-matmul reduce;
            # flushed inside the NEXT chunk's z-groups so the PE never
            # stalls on the tanh/vector chain.
            pending = []

            def flush_pending():
                for acc, pb, pci in pending:
                    pwc = bwidths[pb][pci]
                    pc0 = boffs[pb][pci]
                    pss = psum_s.tile([1, MAXC], f32, tag="pss")
                    nc.tensor.matmul(pss[:1, :pwc], ones_sb[:], acc[:],
                                     start=True, stop=True,
                                     skip_group_check=True)
                    sc_m = sc_pool.tile([1, pwc], f32, tag="sc")
                    off = pb * cap + pc0
                    nc.vector.tensor_tensor(sc_m[:], pss[:1, :pwc],
                                            padc_sb[:, off:off + pwc],
                                            ALU.add)
                    idx = pb * nch + pci
                    nc.scalar.activation(
                        e_rows[pb][:, pc0:pc0 + pwc], sc_m[:], AF.Exp,
                        accum_out=sums[:, idx:idx + 1])
                    if pb == BL - 1:
                        nc.sync.dma_start(eout[pb:pb + 1, pc0:pc0 + pwc],
                                          e_rows[pb][:, pc0:pc0 + pwc])
                    elif pci == nch - 1:
                        nc.sync.dma_start(eout[pb:pb + 1, :], e_rows[pb][:])
                pending.clear()

            for b in range(BL):
                if b == 0:
                    eo_sb = eo_first
                else:
                    eo_sb = eo_pool.tile([P, KP, 2, cap], fp8, tag="eo")
                    nc.gpsimd.dma_start(eo_sb[:], eo8[b])
                order = [(hh, ci) for ci in range(nch) for hh in range(HT)]
                accs = {}
                t8s_m = {}
                pss_m = {}
                for hh, ci in order:
                    c0, wc = boffs[b][ci], bwidths[b][ci]
                    cs = slice(c0, c0 + wc)
                    tail = (b == BL - 1 and ci >= nch - 2)
                    if hh == 0:
                        if tail:
                            pss_m[ci] = psum_s.tile([1, MAXC], f32,
                                                    tag="pss", name="pss_t")
                            t8s_m[ci] = []
                        else:
                            accs[ci] = acc_pool.tile([P, wc], bf16,
                                                     tag="acc", name="acc")
                    zp = psum_z.tile([P, wc], f32, tag="psz")
                    for j in range(KP):
                        nc.tensor.matmul(
                            zp[:], w8_sb[:, hh, j, :, :],
                            eo_sb[:, j, :, cs], start=(j == 0),
                            stop=(j == KP - 1), perf_mode=DR)
                    if hh == 2 and pending:
                        flush_pending()
                    t8 = t_pool.tile([P, wc], bf16, tag="t")
                    nc.scalar.activation(
                        t8[:], zp[:], AF.Tanh, scale=ZS,
                        bias=pre_sb[:, hh * BL + b:hh * BL + b + 1])
                    if tail:
                        pss_t = pss_m[ci]
                        t8s = t8s_m[ci]
                        t8s.append(t8)
                        if hh >= 2:
                            nc.tensor.matmul(
                                pss_t[:1, :wc],
                                vsc8_sb[:, hh - 2:hh - 2 + 1],
                                t8s[hh - 2][:], start=(hh == 2),
                                stop=False, skip_group_check=True)
                    elif hh == 0:
                        nc.vector.tensor_scalar(accs[ci][:], t8[:],
                                                vsc_sb[:, 0:1], None,
                                                ALU.mult)
                    else:
                        tv = t_pool.tile([P, wc], bf16, tag="tv")
                        nc.vector.tensor_scalar(tv[:], t8[:],
                                                vsc_sb[:, hh:hh + 1],
                                                None, ALU.mult)
                        nc.vector.tensor_tensor(accs[ci][:], accs[ci][:],
                                                tv[:], ALU.add)
                    if hh == HT - 1:
                        if tail:
                            for h2 in range(HT - 2, HT):
                                nc.tensor.matmul(
                                    pss_t[:1, :wc], vsc8_sb[:, h2:h2 + 1],
                                    t8s[h2][:], start=False,
                                    stop=(h2 == HT - 1),
                                    skip_group_check=True)
                            sc_m = sc_pool.tile([1, wc], f32, tag="sc")
                            off = b * cap + c0
                            nc.vector.tensor_tensor(sc_m[:], pss_t[:1, :wc],
                                                    padc_sb[:, off:off + wc],
                                                    ALU.add)
                            idx = b * nch + ci
                            nc.scalar.activation(
                                e_rows[b][:, c0:c0 + wc], sc_m[:], AF.Exp,
                                accum_out=sums[:, idx:idx + 1])
                            nc.sync.dma_start(
                                eout[b:b + 1, c0:c0 + wc],
                                e_rows[b][:, c0:c0 + wc])
                        else:
                            pending.append((accs[ci], b, ci))
            flush_pending()
            nc.sync.dma_start(sout[:, :], sums[:])

    nc.compile()
    return nc


def _get_nc(cap=1072):
    if cap not in _compiled:
        _compiled[cap] = _build(cap)
    return _compiled[cap]


def _prep(hidden, encoder_outputs, encoder_mask, W, b, v):
    """Host-side packing/quantization. Returns (in_maps, scatter_info)."""
    import ml_dtypes

    bf16 = ml_dtypes.bfloat16
    f8 = ml_dtypes.float8_e4m3

    hidden = np.asarray(hidden, dtype=np.float32)
    eo = np.asarray(encoder_outputs, dtype=np.float32)      # [S, B, H]
    W = np.asarray(W, dtype=np.float32)
    bias = np.asarray(b, dtype=np.float32)
    v = np.asarray(v, dtype=np.float32)
    mask = np.asarray(encoder_mask).reshape(B, S)

    Wh, We = W[:, :H], W[:, H:]

    w8 = (We * SW).astype(f8)
    w8f = w8.astype(np.float32)
    vb = v.astype(bf16).astype(np.float32)
    u = (We.T @ v).astype(np.float32)            # exact linear weights
    u8 = (w8f / SW).T @ vb                       # device linear weights

    pre = hidden @ Wh.T + bias                   # [B, H] exact hidden path

    # per-batch unmasked indices, uniform padded capacity
    idxs = [np.nonzero(mask[gb] == 0)[0] for gb in range(B)]
    ns = [len(ix) for ix in idxs]
    cap = max(8, -(-max(max(ns), 1) // 8) * 8)

    # stationary layout: k = j*256 + i*128 + p
    w8st = np.ascontiguousarray(
        w8.T.reshape(KP, 2, P, HT, P).transpose(2, 3, 0, 1, 4))
    vsc = np.ascontiguousarray(
        v.astype(bf16).astype(np.float32).reshape(HT, P).T)

    in_maps = []
    for c in range(NCORES):
        eo8c = np.zeros((BL, P, KP, 2, cap), dtype=f8)
        padc = np.zeros((BL, cap), dtype=np.float32)
        for bl in range(BL):
            gb = c * BL + bl
            ix = idxs[gb]
            n = len(ix)
            ecols = np.ascontiguousarray(eo[ix, gb, :].T)   # [H, n]
            e8 = (ecols * SE).astype(f8)
            eo8c[bl, :, :, :, :n] = e8.reshape(
                KP, 2, P, n).transpose(2, 0, 1, 3)
            # exact linear correction of the fp8 z-path
            padc[bl, :n] = u @ ecols - (u8 @ e8.astype(np.float32)) / SE
            padc[bl, n:] = -1e30

        pre_c = pre[c * BL:(c + 1) * BL]                    # [BL, H]
        pre_r = np.ascontiguousarray(
            pre_c.reshape(BL, HT, P).transpose(2, 1, 0).reshape(P, HT * BL))
        in_maps.append({
            "eo8": eo8c,
            "w8st": w8st,
            "vsc": vsc,
            "prer": pre_r,
            "padc": padc.reshape(1, BL * cap),
        })
    return in_maps, (idxs, ns, cap)


def run(hidden, encoder_outputs, encoder_mask, W, b, v, trace=False):
    from concourse.bass_utils import run_bass_kernel_spmd

    in_maps, (idxs, ns, cap) = _prep(
        hidden, encoder_outputs, encoder_mask, W, b, v)
    nc = _get_nc(cap)
    res = run_bass_kernel_spmd(nc, in_maps, core_ids=list(range(NCORES)),
                               trace=trace)
    nch = len(_chunks(cap))
    full = np.zeros((B, S), dtype=np.float32)
    for c in range(NCORES):
        e = res.results[c]["eout"]
        sm = res.results[c]["sout"].reshape(BL, nch).sum(axis=1)
        for bl in range(BL):
            gb = c * BL + bl
            if ns[gb] == 0:
                full[gb, :] = 1.0 / S     # all masked: softmax is uniform
            else:
                full[gb, idxs[gb]] = e[bl, :ns[gb]] / sm[bl]
    return full.reshape(B, 1, S), res


def kernel(hidden, encoder_outputs, encoder_mask, W, b, v):
    out, _ = run(hidden, encoder_outputs, encoder_mask, W, b, v, trace=False)
    return out
